# revision 2
# baseline (speedup 1.0000x reference)
"""Distributed attention kernel for 8 TRN2 NeuronCores.

Sharding: core c -> (batch b = c//2, head-half hh = c%2).  Each core computes
LN(x_b) -> q for its 8 heads over all 2048 rows, k/v over the COMPACTED
visible key rows (context-masked keys contribute exactly exp(-inf)=0, so the
host drops them; the null key is appended) -> l2norm cosine attention ->
partial out @ wo[head-slice].  Host sums the two partial outputs per batch.

On-core dataflow (activations feature-on-partition, bf16 matmul operands):
  xT/xvT bf16 -> LN stats via ones-matmul (f32 psum) -> normalize in place ->
  qT/kT resident in SBUF, v natural [key, head*65] with a ones-augment column
  -> per head-pair m: simT = k_chunk @ qhat^T (psum f32),
  exp(8*rsnk*sim + maskbias) fused on ScalarE -> bf16, AV matmul accumulates
  [out | denom] via the ones column -> divide -> out @ wo.
"""

import sys

sys.path.insert(0, "/opt/trn_rl_repo")

import numpy as np  # noqa: E402
import ml_dtypes  # noqa: E402

import concourse.bacc as bacc  # noqa: E402
import concourse.bass as bass  # noqa: E402
import concourse.tile as tile  # noqa: E402
from concourse import mybir  # noqa: E402
from concourse.bass_utils import run_bass_kernel_spmd  # noqa: E402

BF = ml_dtypes.bfloat16
F32 = mybir.dt.float32
BF16 = mybir.dt.bfloat16
AF = mybir.ActivationFunctionType

P = 128
N = 2048          # query rows per batch
D = 1024          # model dim
HC = 8            # heads per core
IC = 512          # inner dim per core
NEG = -1.0e4
EPS_LN = 1e-5
EPS_L2 = 1e-12
SCALE = 8.0

KEYSC = 1408      # 11*128 key slots: [0:KV) visible-compacted, KV null, pads
KV = KEYSC - P    # 1280 padded visible rows
KC = KEYSC // P   # 11


def _chunks(total, step=512):
    return [(c, min(c + step, total)) for c in range(0, total, step)]


def build_nc(keysc=KEYSC):
    kv = keysc - P
    kcn = keysc // P
    nc = bacc.Bacc(None, target_bir_lowering=False)

    xT_d = nc.dram_tensor("xT", [D, N], BF16, kind="ExternalInput")
    xvT_d = nc.dram_tensor("xvT", [D, kv], BF16, kind="ExternalInput")
    wq_d = nc.dram_tensor("wq", [D, IC], BF16, kind="ExternalInput")
    wk_d = nc.dram_tensor("wk", [D, IC], BF16, kind="ExternalInput")
    wv_d = nc.dram_tensor("wv", [D, IC], BF16, kind="ExternalInput")
    wo_d = nc.dram_tensor("wo", [IC, D], BF16, kind="ExternalInput")
    nullk_d = nc.dram_tensor("nullk", [P, 4], BF16, kind="ExternalInput")
    vlast_d = nc.dram_tensor("vlast", [P, HC * 65], BF16, kind="ExternalInput")
    mask_d = nc.dram_tensor("maskcol", [P, kcn], F32, kind="ExternalInput")
    id2_d = nc.dram_tensor("id2", [P, 2], F32, kind="ExternalInput")
    out_d = nc.dram_tensor("out", [N, D], F32, kind="ExternalOutput")

    with tile.TileContext(nc) as tc:
        with (
            tc.tile_pool(name="consts", bufs=1) as cns,
            tc.tile_pool(name="small", bufs=2) as sml,
            tc.tile_pool(name="rep", bufs=4) as repp,
            tc.tile_pool(name="scratch", bufs=2) as scr,
            tc.tile_pool(name="qkv", bufs=1) as qkv,
            tc.tile_pool(name="ps", bufs=4, space="PSUM") as ps,
            tc.tile_pool(name="dram", bufs=1, space="DRAM") as drp,
        ):
            ones1b = cns.tile([P, 1], BF16)
            nc.vector.memset(ones1b, 1.0)
            blkdiag = cns.tile([P, 2], BF16)
            nc.vector.memset(blkdiag, 0.0)
            nc.vector.memset(blkdiag[0:64, 0:1], 1.0)
            nc.vector.memset(blkdiag[64:128, 1:2], 1.0)
            ident2 = cns.tile([P, 2], F32)
            nc.sync.dma_start(out=ident2, in_=id2_d[:, :])
            eps_col = cns.tile([P, 1], F32)
            nc.vector.memset(eps_col, EPS_LN)
            sc_col = cns.tile([P, 1], F32)
            nc.vector.memset(sc_col, 1.0 / float(D * D))
            maskc = cns.tile([P, kcn], F32)
            nc.sync.dma_start(out=maskc, in_=mask_d[:, :])
            nullk_sb = cns.tile([P, 4], BF16)
            nc.sync.dma_start(out=nullk_sb, in_=nullk_d[:, :])
            zpadb = cns.tile([P, P], BF16, name="zpadb")
            nc.vector.memset(zpadb, 0.0)

            oT_dr = drp.tile([P, 4, N], BF16)
            bnc_dr = drp.tile([2, N], BF16)

            qT = qkv.tile([P, 4, N], BF16)
            kT = qkv.tile([P, 4, keysc], BF16)
            v_sb = qkv.tile([P, kcn, HC * 65], BF16)

            def ln_stats(xt, ncols, bnc, tg):
                """Emit stats for xt [P, 8, ncols]; bounce s/ms rows to bnc."""
                sum_sb = sml.tile([1, N], F32, tag="sum", bufs=1, name="sum_sb")
                sumsq_sb = sml.tile([1, N], F32, tag="sumsq",
                                    bufs=1, name="sumsq_sb")
                for (c0, c1) in _chunks(ncols):
                    w = c1 - c0
                    sA = ps.tile([1, 512], F32, tag="ps", name="sA")
                    sB = ps.tile([1, 512], F32, tag="ps", name="sB")
                    for f in range(8):
                        xc = xt[:, f, c0:c1]
                        sq = scr.tile([P, 512], BF16, tag="sq", name="sq")
                        nc.vector.tensor_mul(sq[:, 0:w], xc, xc)
                        nc.tensor.matmul(sA[:, 0:w], ones1b, xc,
                                         start=(f == 0), stop=(f == 7))
                        nc.tensor.matmul(sB[:, 0:w], ones1b, sq[:, 0:w],
                                         start=(f == 0), stop=(f == 7))
                    nc.vector.tensor_copy(sum_sb[:, c0:c1], sA[:, 0:w])
                    nc.vector.tensor_copy(sumsq_sb[:, c0:c1], sB[:, 0:w])
                cs = slice(0, ncols)
                A1 = sml.tile([1, N], F32, tag="t1", bufs=1, name="A1")
                nc.vector.tensor_mul(A1[:, cs], sum_sb[:, cs], sum_sb[:, cs])
                nc.vector.tensor_scalar_mul(sumsq_sb[:, cs], sumsq_sb[:, cs],
                                            float(D))
                nc.vector.tensor_sub(sumsq_sb[:, cs], sumsq_sb[:, cs],
                                     A1[:, cs])
                nc.scalar.activation(A1[:, cs], sumsq_sb[:, cs], AF.Sqrt,
                                     scale=sc_col[0:1, :],
                                     bias=eps_col[0:1, :])
                nc.vector.reciprocal(sumsq_sb[:, cs], A1[:, cs])   # s
                nc.vector.tensor_mul(A1[:, cs], sum_sb[:, cs], sumsq_sb[:, cs])
                nc.vector.tensor_scalar_mul(A1[:, cs], A1[:, cs],
                                            1.0 / float(D))        # ms
                rows_s = sml.tile([1, N], BF16, tag="rowsbf",
                                  bufs=1, name="rows_s")
                rows_m = sml.tile([1, N], BF16, tag="rowsbf2",
                                  bufs=1, name="rows_m")
                nc.vector.tensor_copy(rows_s[0:1, cs], sumsq_sb[0:1, cs])
                nc.vector.tensor_copy(rows_m[0:1, cs], A1[0:1, cs])
                nc.sync.dma_start(out=bnc[0:1, cs], in_=rows_s[0:1, cs])
                nc.sync.dma_start(out=bnc[1:2, cs], in_=rows_m[0:1, cs])

            def ln_apply(xt, ncols, bnc):
                cs = slice(0, ncols)
                rep_s = repp.tile([P, N], BF16, tag="rep", name="rep_s")
                rep_m = repp.tile([P, N], BF16, tag="rep", name="rep_m")
                for (i, r) in ((0, rep_s), (1, rep_m)):
                    src = bnc[i, cs]
                    nc.sync.dma_start(
                        out=r[:, cs],
                        in_=bass.AP(tensor=src.tensor, offset=src.offset,
                                    ap=[[0, P]] + src.ap))
                for f in range(8):
                    nc.vector.tensor_mul(xt[:, f, :], xt[:, f, :],
                                         rep_s[:, cs])
                    nc.vector.tensor_sub(xt[:, f, :], xt[:, f, :],
                                         rep_m[:, cs])

            # ------------- phases A+B: LN + q/k/v projections -------------
            with (
                tc.tile_pool(name="xp", bufs=1) as xp,
                tc.tile_pool(name="xvp", bufs=1) as xvp,
            ):
                xT = xp.tile([P, 8, N], BF16)
                for f in range(8):
                    nc.sync.dma_start(
                        out=xT[:, f, :],
                        in_=xT_d.rearrange("(f p) r -> f p r", p=P)[f, :, :])
                xvT = xvp.tile([P, 8, kv], BF16)
                for f in range(8):
                    nc.sync.dma_start(
                        out=xvT[:, f, :],
                        in_=xvT_d.rearrange("(f p) r -> f p r", p=P)[f, :, :])
                bncv = drp.tile([2, N], BF16, name="bncv")
                ln_stats(xT, N, bnc_dr, "a")
                ln_stats(xvT, kv, bncv, "b")
                ln_apply(xT, N, bnc_dr)
                ln_apply(xvT, kv, bncv)
                with tc.tile_pool(name="wp2", bufs=1) as wp2:
                    w_sb = wp2.tile([P, 8, IC], BF16, tag="w", name="wq_sb")
                    nc.sync.dma_start(
                        out=w_sb, in_=wq_d.rearrange("(f p) j -> p f j", p=P))
                    for m in range(4):
                        for (c0, c1) in _chunks(N):
                            qp = ps.tile([P, 512], F32, tag="ps", name="qp")
                            for f in range(8):
                                nc.tensor.matmul(
                                    qp, w_sb[:, f, m * P:(m + 1) * P],
                                    xT[:, f, c0:c1],
                                    start=(f == 0), stop=(f == 7))
                            nc.scalar.copy(qT[:, m, c0:c1], qp)

                    w_sbk = wp2.tile([P, 8, IC], BF16, tag="w", name="wk_sb")
                    nc.sync.dma_start(
                        out=w_sbk, in_=wk_d.rearrange("(f p) j -> p f j", p=P))
                    for m in range(4):
                        for (c0, c1) in _chunks(kv):
                            kp = ps.tile([P, 512], F32, tag="ps", name="kp")
                            for f in range(8):
                                nc.tensor.matmul(
                                    kp[:, 0:c1 - c0],
                                    w_sbk[:, f, m * P:(m + 1) * P],
                                    xvT[:, f, c0:c1],
                                    start=(f == 0), stop=(f == 7))
                            nc.scalar.copy(kT[:, m, c0:c1], kp[:, 0:c1 - c0])
                    nc.sync.dma_start(
                        out=kT[:, :, kv:kv + 1],
                        in_=nullk_sb.rearrange("p (m o) -> p m o", o=1))
                    for m in range(4):
                        nc.sync.dma_start(out=kT[:, m, kv + 1:keysc],
                                          in_=zpadb[:, 0:keysc - kv - 1])

                    w_sbv = wp2.tile([P, 8, IC], BF16, tag="w", name="wv_sb")
                    nc.sync.dma_start(
                        out=w_sbv, in_=wv_d.rearrange("(f p) j -> p f j", p=P))
                    for rt in range(kv // P):
                        vp = ps.tile([P, 512], F32, tag="ps", name="vp")
                        for f in range(8):
                            nc.tensor.matmul(
                                vp, xvT[:, f, rt * P:(rt + 1) * P],
                                w_sbv[:, f, :],
                                start=(f == 0), stop=(f == 7))
                        nc.scalar.copy(
                            v_sb[:, rt, :].rearrange(
                                "p (h c) -> p h c", c=65)[:, :, 0:64],
                            vp.rearrange("p (h c) -> p h c", c=64))
                    nc.vector.memset(
                        v_sb[:, 0:kv // P, :].rearrange(
                            "p t (h c) -> p t h c", c=65)[:, :, :, 64:65], 1.0)
                    nc.sync.dma_start(out=v_sb[:, kcn - 1, :],
                                      in_=vlast_d[:, :])

            # ---------------- attention ----------------
            with (
                tc.tile_pool(name="expp", bufs=3) as expp,
                tc.tile_pool(name="omp", bufs=2) as omp,
            ):
                # ---- hoisted prologue: l2 norms for all m ----
                nstq = sml.tile([P, N], F32, tag="nstq", bufs=1, name="nstq")
                nstk = sml.tile([P, N], F32, tag="nstk", bufs=1, name="nstk")
                nstk2 = sml.tile([2, N], F32, tag="nstk2", bufs=1, name="nstk2")
                nc.vector.memset(nstq, 1.0)
                nc.vector.memset(nstk, 1.0)
                nc.vector.memset(nstk2, 1.0)
                for m in range(4):
                    sqq = scr.tile([P, N], BF16, tag="sqbig", bufs=2,
                                   name="sqq")
                    nc.vector.tensor_mul(sqq, qT[:, m, :], qT[:, m, :])
                    for (c0, c1) in _chunks(N):
                        t = ps.tile([2, 512], F32, tag="ps", name="tq")
                        nc.tensor.matmul(t, blkdiag, sqq[:, c0:c1],
                                         start=True, stop=True)
                        nc.vector.tensor_copy(
                            nstq[32 * m:32 * m + 2, c0:c1], t)
                    sqk = scr.tile([P, keysc], BF16, tag="sqbig", bufs=2,
                                   name="sqk")
                    nc.vector.tensor_mul(sqk, kT[:, m, :], kT[:, m, :])
                    for (c0, c1) in _chunks(keysc):
                        t = ps.tile([2, 512], F32, tag="ps", name="tk")
                        nc.tensor.matmul(t[:, 0:c1 - c0], blkdiag,
                                         sqk[:, c0:c1], start=True, stop=True)
                        if m < 3:
                            nc.vector.tensor_copy(
                                nstk[32 * m:32 * m + 2, c0:c1],
                                t[:, 0:c1 - c0])
                        else:
                            nc.vector.tensor_copy(
                                nstk2[0:2, c0:c1], t[:, 0:c1 - c0])
                for t_ in (nstq, nstk, nstk2):
                    nc.scalar.activation(t_, t_, AF.Sqrt)
                    nc.vector.tensor_scalar_max(t_, t_, EPS_L2)
                    nc.vector.reciprocal(t_, t_)
                nq_bf = sml.tile([P, N], BF16, tag="nqbf", bufs=1, name="nq_bf")
                nc.vector.tensor_copy(nq_bf, nstq)
                bncp = drp.tile([8, N], BF16, name="bncp")
                for m in range(4):
                    for h2 in range(2):
                        r = 32 * m + h2
                        nc.sync.dma_start(out=bncp[2 * m + h2:2 * m + h2 + 1, :],
                                          in_=nq_bf[r:r + 1, :])
                for m in range(4):
                    repq = repp.tile([P, N], BF16, tag="rep", name="repq")
                    for h2 in range(2):
                        src = bncp[2 * m + h2, :]
                        nc.sync.dma_start(
                            out=repq[64 * h2:64 * (h2 + 1), :],
                            in_=bass.AP(tensor=src.tensor, offset=src.offset,
                                        ap=[[0, 64]] + src.ap))
                    nc.vector.tensor_mul(qT[:, m, :], qT[:, m, :], repq)
                rkts = []
                for m in range(4):
                    rkT = sml.tile([P, kcn, 2], F32, tag="rkT", bufs=4,
                                   name=f"rkT{m}")
                    for kc in range(kcn):
                        t = ps.tile([P, 2], F32, tag="ps", name="tt")
                        if m < 3:
                            nc.tensor.transpose(
                                t,
                                nstk[32 * m:32 * m + 2, kc * P:(kc + 1) * P],
                                ident2[32 * m:32 * m + 2, :])
                        else:
                            nc.tensor.transpose(
                                t, nstk2[0:2, kc * P:(kc + 1) * P],
                                ident2[0:2, :])
                        nc.vector.tensor_copy(rkT[:, kc, :], t)
                    nc.vector.tensor_scalar_mul(rkT, rkT, SCALE)
                    rkts.append(rkT)

                for m in range(4):
                    rkT = rkts[m]
                    om_t = [omp.tile([64, N], BF16, tag="om", name=f"om{i}")
                            for i in range(2)]
                    bnc2 = drp.tile([2, N], BF16, tag="bnc2", bufs=4,
                                    name=f"bnc2_{m}")
                    for rc in range(2):
                        ops = [ps.tile([65, 1024], F32, tag="ps",
                                       name=f"avp{i}") for i in range(2)]
                        for kc in range(kcn):
                            for h2 in range(2):
                                sim = ps.tile([P, 1024], F32, tag="ps",
                                              name="sim")
                                for nh in range(2):
                                    r0 = rc * 1024 + nh * 512
                                    nc.tensor.matmul(
                                        sim[:, nh * 512:(nh + 1) * 512],
                                        kT[64 * h2:64 * (h2 + 1),
                                           m, kc * P:(kc + 1) * P],
                                        qT[64 * h2:64 * (h2 + 1),
                                           m, r0:r0 + 512],
                                        start=True, stop=True)
                                e = expp.tile([P, 1024], BF16, tag="e",
                                              name="e")
                                nc.scalar.activation(
                                    e, sim, AF.Exp,
                                    bias=maskc[:, kc:kc + 1],
                                    scale=rkT[:, kc, h2:h2 + 1])
                                for nh in range(2):
                                    nc.tensor.matmul(
                                        ops[h2][:, nh * 512:(nh + 1) * 512],
                                        v_sb[:, kc,
                                             (2 * m + h2) * 65:
                                             (2 * m + h2 + 1) * 65],
                                        e[:, nh * 512:(nh + 1) * 512],
                                        start=(kc == 0), stop=(kc == kcn - 1))
                        for h2 in range(2):
                            dnr = omp.tile([65, 1024], BF16, tag="dn",
                                           bufs=2, name=f"dnr{h2}")
                            with nc.allow_low_precision("bf16 denom"):
                                nc.vector.reciprocal(dnr[64:65, :],
                                                     ops[h2][64:65, :])
                            nc.scalar.copy(
                                om_t[h2][:, rc * 1024:(rc + 1) * 1024],
                                ops[h2][0:64, :])
                            nc.sync.dma_start(
                                out=bnc2[h2:h2 + 1,
                                         rc * 1024:(rc + 1) * 1024],
                                in_=dnr[64:65, :])
                    for h2 in range(2):
                        src = bnc2[h2, :]
                        repd = repp.tile([64, N], BF16, tag="rep", name="repd")
                        nc.sync.dma_start(
                            out=repd,
                            in_=bass.AP(tensor=src.tensor, offset=src.offset,
                                        ap=[[0, 64]] + src.ap))
                        nc.vector.tensor_mul(om_t[h2], om_t[h2], repd)
                        nc.sync.dma_start(
                            out=oT_dr[64 * h2:64 * (h2 + 1), m, :],
                            in_=om_t[h2])

            # ---------------- output projection ----------------
            with (
                tc.tile_pool(name="wop", bufs=1) as wop,
                tc.tile_pool(name="lhp", bufs=3) as lhp,
            ):
                wo_sb = wop.tile([P, 4, D], BF16)
                nc.sync.dma_start(
                    out=wo_sb, in_=wo_d.rearrange("(m p) j -> p m j", p=P))
                for rt in range(16):
                    lh = lhp.tile([P, 4, P], BF16, tag="lh", name="lh")
                    nc.sync.dma_start(out=lh,
                                      in_=oT_dr[:, :, rt * P:(rt + 1) * P])
                    for n2 in range(2):
                        op = ps.tile([P, 512], F32, tag="ps", name="op")
                        for m in range(4):
                            nc.tensor.matmul(
                                op, lh[:, m, :],
                                wo_sb[:, m, n2 * 512:(n2 + 1) * 512],
                                start=(m == 0), stop=(m == 3))
                        stg = scr.tile([P, 512], F32, tag="stg", name="stg")
                        nc.scalar.copy(stg, op)
                        nc.sync.dma_start(
                            out=out_d[rt * P:(rt + 1) * P,
                                      n2 * 512:(n2 + 1) * 512],
                            in_=stg)

    nc.finalize()
    return nc


_NC = {}


def _get_nc(keysc=KEYSC):
    if keysc not in _NC:
        _NC[keysc] = build_nc(keysc)
    return _NC[keysc]


def _shards(x, context_mask, gamma, wq, wkv, null_kv, wo, keysc):
    kv = keysc - P
    kcn = keysc // P
    x = np.asarray(x, np.float32)
    gamma = np.asarray(gamma, np.float32)
    wq_g = (np.asarray(wq, np.float32) * gamma[:, None]).astype(BF)
    wkv_g = np.asarray(wkv, np.float32) * gamma[:, None]
    wk_g = wkv_g[:, :D].astype(BF)
    wv_g = wkv_g[:, D:].astype(BF)
    wo = np.asarray(wo, np.float32)
    null_kv = np.asarray(null_kv, np.float32)
    cm = np.asarray(context_mask)

    maps = []
    for c in range(8):
        b, hh = c // 2, c % 2
        sl = slice(hh * IC, (hh + 1) * IC)
        heads = np.arange(HC) + hh * HC
        nk = null_kv[0][heads, 0, :]
        nv = null_kv[1][heads, 0, :]
        nullk = np.ascontiguousarray(
            nk.reshape(4, 2, 64).transpose(1, 2, 0).reshape(P, 4))
        vlast = np.zeros((P, HC * 65), np.float32)
        vlast[:, 64::65] = 1.0
        for h in range(HC):
            vlast[0, h * 65:h * 65 + 64] = nv[h]
        vis = np.flatnonzero(cm[b])
        nvis = len(vis)
        xv = np.zeros((kv, D), np.float32)
        xv[:nvis] = x[b][vis]
        bias = np.full((keysc,), NEG, np.float32)
        bias[:nvis] = 0.0
        bias[kv] = 0.0          # null key always visible
        maskcol = np.ascontiguousarray(bias.reshape(kcn, P).T)
        maps.append({
            "xT": np.ascontiguousarray(x[b].T).astype(BF),
            "xvT": np.ascontiguousarray(xv.T).astype(BF),
            "wq": np.ascontiguousarray(wq_g[:, sl]),
            "wk": np.ascontiguousarray(wk_g[:, sl]),
            "wv": np.ascontiguousarray(wv_g[:, sl]),
            "wo": np.ascontiguousarray(wo[sl, :]).astype(BF),
            "nullk": nullk.astype(BF),
            "vlast": vlast.astype(BF),
            "maskcol": maskcol,
            "id2": np.tile(np.eye(2, dtype=np.float32), (64, 1)),
        })
    return maps


def kernel(x, context_mask, gamma, wq, wkv, null_kv, q_scale, k_scale, wo,
           _trace=False):
    cm = np.asarray(context_mask)
    max_vis = int(cm.sum(axis=1).max())
    keysc = KEYSC
    if max_vis > KV:
        keysc = ((max_vis + P) // P + 1) * P   # room for null + padding
    nc = _get_nc(keysc)
    maps = _shards(x, context_mask, gamma, wq, wkv, null_kv, wo, keysc)
    res = run_bass_kernel_spmd(nc, maps, core_ids=list(range(8)),
                               trace=_trace,
                               tmpdir="/tmp/bass_trace" if _trace else None)
    outs = [np.asarray(res.results[c]["out"], np.float32) for c in range(8)]
    full = np.stack([outs[2 * b] + outs[2 * b + 1] for b in range(4)])
    if _trace:
        kernel.last_exec_time_ns = res.exec_time_ns
    return full



# revision 11
# speedup vs baseline: 1.4591x; 1.4591x over previous
"""Distributed attention kernel for 8 TRN2 NeuronCores (v2).

Sharding: core c -> (batch b = c//2, head-half hh = c%2).  Each core computes
LN(x_b) for all 2048 rows, q for its 8 heads, k/v over the first KEYSC-1
rows of a HOST-PERMUTED x (visible rows first, masked rows after; the output
is un-permuted on the host), l2norm cosine attention with the null k/v
appended at slot KEYSC-1, and a partial out @ wo[head-slice].  Host sums the
two partial outputs per batch.

v2 changes vs v1 (779us baseline):
  - single LN pass over the permuted x (no separate compacted xvT tensor)
  - key space 1408 -> 1152 slots (null key folded into the last chunk)
  - all 1/x via reciprocal_approx_fast (DVE iterative reciprocal was 164us
    and serialized the PE into its cold-clock regime)
  - k-norms (and SCALE, k_scale/q_scale) pre-folded into kT/qT so the exp
    activation needs no per-key scale operand
  - softmax division deferred past the attention loop; output stays in SBUF
  - LN -> projections pipelined per 512-column chunk to keep the PE dense
"""

import sys

sys.path.insert(0, "/opt/trn_rl_repo")

import numpy as np  # noqa: E402
import ml_dtypes  # noqa: E402

import concourse.bacc as bacc  # noqa: E402
import concourse.bass as bass  # noqa: E402
import concourse.tile as tile  # noqa: E402
from concourse import mybir  # noqa: E402
from concourse.bass_utils import run_bass_kernel_spmd  # noqa: E402

BF = ml_dtypes.bfloat16
F32 = mybir.dt.float32
BF16 = mybir.dt.bfloat16
AF = mybir.ActivationFunctionType
MUL = mybir.AluOpType.mult

P = 128
N = 2048          # query rows per batch
D = 1024          # model dim
HC = 8            # heads per core
IC = 512          # inner dim per core
NEG = -1.0e4
EPS_LN = 1e-5
EPS_L2 = 1e-12
SCALE = 8.0

KEYSC = 1152      # key slots: [0:nvis) visible, pads, null at KEYSC-1


def _chunks(total, step=512):
    return [(c, min(c + step, total)) for c in range(0, total, step)]


def build_nc(keysc=KEYSC):
    kcn = keysc // P
    nc = bacc.Bacc(None, target_bir_lowering=False)

    xT_d = nc.dram_tensor("xT", [D, N], BF16, kind="ExternalInput")
    wq_d = nc.dram_tensor("wq", [D, IC], BF16, kind="ExternalInput")
    wk_d = nc.dram_tensor("wk", [D, IC], BF16, kind="ExternalInput")
    wv_d = nc.dram_tensor("wv", [D, IC], BF16, kind="ExternalInput")
    wo_d = nc.dram_tensor("wo", [IC, D], BF16, kind="ExternalInput")
    nullk_d = nc.dram_tensor("nullk", [P, 4], BF16, kind="ExternalInput")
    nullv_d = nc.dram_tensor("nullv", [1, HC * 65], BF16, kind="ExternalInput")
    mask_d = nc.dram_tensor("maskcol", [P, kcn], F32, kind="ExternalInput")
    qks_d = nc.dram_tensor("qks", [P, 1], F32, kind="ExternalInput")
    out_d = nc.dram_tensor("out", [N, D], F32, kind="ExternalOutput")

    with tile.TileContext(nc) as tc:
        with (
            tc.tile_pool(name="consts", bufs=1) as cns,
            tc.tile_pool(name="qkv", bufs=1) as qkv,
            tc.tile_pool(name="wop", bufs=1) as wop,
            tc.tile_pool(name="rep", bufs=4) as repp,
            tc.tile_pool(name="dram", bufs=1, space="DRAM") as drp,
        ):
            ones1b = cns.tile([P, 1], BF16)
            nc.vector.memset(ones1b, 1.0)
            blkdiag = cns.tile([P, 2], BF16)
            nc.vector.memset(blkdiag, 0.0)
            nc.vector.memset(blkdiag[0:64, 0:1], 1.0)
            nc.vector.memset(blkdiag[64:128, 1:2], 1.0)
            maskc = cns.tile([P, kcn], F32)
            nc.sync.dma_start(out=maskc, in_=mask_d[:, :])
            nullk_sb = cns.tile([P, 4], BF16)
            nc.sync.dma_start(out=nullk_sb, in_=nullk_d[:, :])
            qks_sb = cns.tile([P, 1], F32)
            nc.sync.dma_start(out=qks_sb, in_=qks_d[:, :])
            eps_col = cns.tile([P, 1], F32)
            nc.vector.memset(eps_col, EPS_LN)
            sc_col = cns.tile([P, 1], F32)
            nc.vector.memset(sc_col, 1.0 / float(D * D))
            eps2_col = cns.tile([P, 1], F32)
            nc.vector.memset(eps2_col, EPS_L2 * EPS_L2)

            qT = qkv.tile([P, 4, N], BF16)
            kT = qkv.tile([P, 4, keysc], BF16)
            v_sb = qkv.tile([P, kcn, HC * 65], BF16)
            oT = qkv.tile([P, 4, N], BF16)

            wo_sb = wop.tile([P, 4, D], BF16)
            nc.sync.dma_start(
                out=wo_sb, in_=wo_d.rearrange("(m p) j -> p m j", p=P))

            nc.vector.memset(
                v_sb.rearrange("p t (h c) -> p t h c", c=65)[:, :, :, 64:65],
                1.0)

            bnc_dr = drp.tile([2, N], BF16, name="bnc")
            den_dr = drp.tile([8, N], BF16, name="den")
            rec_dr = drp.tile([8, N], BF16, name="rec")

            # ---------- phase A: LN + q/k/v projections, chunk-pipelined ----
            with (
                tc.tile_pool(name="xp", bufs=1) as xp,
                tc.tile_pool(name="wp", bufs=1) as wp,
                tc.tile_pool(name="asml", bufs=2) as sml,
                tc.tile_pool(name="ascr", bufs=2) as scr,
                tc.tile_pool(name="lnps", bufs=2, space="PSUM") as lnps,
                tc.tile_pool(name="pjps", bufs=3, space="PSUM") as pjps,
            ):
                wq_sb = wp.tile([P, 8, IC], BF16, tag="wq")
                wk_sb = wp.tile([P, 8, IC], BF16, tag="wk")
                wv_sb = wp.tile([P, 8, IC], BF16, tag="wv")
                nc.sync.dma_start(
                    out=wq_sb, in_=wq_d.rearrange("(f p) j -> p f j", p=P))
                nc.sync.dma_start(
                    out=wk_sb, in_=wk_d.rearrange("(f p) j -> p f j", p=P))
                nc.sync.dma_start(
                    out=wv_sb, in_=wv_d.rearrange("(f p) j -> p f j", p=P))
                xT = xp.tile([P, 8, N], BF16)
                xre = xT_d.rearrange("(f p) r -> f p r", p=P)
                for ci, (c0, c1) in enumerate(_chunks(N)):
                    w = c1 - c0
                    for f in range(8):
                        nc.sync.dma_start(out=xT[:, f, c0:c1],
                                          in_=xre[f, :, c0:c1])
                    sA = lnps.tile([1, 512], F32, tag="lA", name="sA")
                    sB = lnps.tile([1, 512], F32, tag="lB", name="sB")
                    for f in range(8):
                        xc = xT[:, f, c0:c1]
                        sq = scr.tile([P, 512], BF16, tag="sq", name="sq")
                        nc.vector.tensor_mul(sq[:, 0:w], xc, xc)
                        nc.tensor.matmul(sA[:, 0:w], ones1b, xc,
                                         start=(f == 0), stop=(f == 7))
                        nc.tensor.matmul(sB[:, 0:w], ones1b, sq[:, 0:w],
                                         start=(f == 0), stop=(f == 7))
                    # s = 1/sqrt(var+eps); ms = mean*s  (rows bf16)
                    sumr = sml.tile([1, 512], F32, tag="sumr", name="sumr")
                    nc.vector.tensor_copy(sumr[:, 0:w], sA[:, 0:w])
                    a1 = sml.tile([1, 512], F32, tag="a1", name="a1")
                    nc.vector.tensor_mul(a1[:, 0:w], sumr[:, 0:w],
                                         sumr[:, 0:w])
                    t1 = sml.tile([1, 512], F32, tag="t1", name="t1")
                    nc.vector.tensor_scalar(t1[:, 0:w], sB[:, 0:w],
                                            float(D), None, MUL)
                    nc.vector.tensor_sub(t1[:, 0:w], t1[:, 0:w], a1[:, 0:w])
                    sd = sml.tile([1, 512], F32, tag="sd", name="sd")
                    nc.scalar.activation(sd[:, 0:w], t1[:, 0:w], AF.Sqrt,
                                         scale=sc_col[0:1, :],
                                         bias=eps_col[0:1, :])
                    sf = sml.tile([1, 512], F32, tag="sf", name="sf")
                    nc.vector.reciprocal_approx_fast(sf[:, 0:w], sd[:, 0:w])
                    row_s = sml.tile([1, 512], BF16, tag="rows", name="row_s")
                    nc.vector.tensor_copy(row_s[:, 0:w], sf[:, 0:w])
                    nc.vector.tensor_mul(a1[:, 0:w], sumr[:, 0:w],
                                         sf[:, 0:w])
                    row_m = sml.tile([1, 512], BF16, tag="rowm", name="row_m")
                    nc.vector.tensor_scalar(row_m[:, 0:w], a1[:, 0:w],
                                            1.0 / float(D), None, MUL)
                    nc.sync.dma_start(out=bnc_dr[0:1, c0:c1],
                                      in_=row_s[:, 0:w])
                    nc.sync.dma_start(out=bnc_dr[1:2, c0:c1],
                                      in_=row_m[:, 0:w])
                    rep_s = repp.tile([P, 512], BF16, tag="rep", name="rep_s")
                    rep_m = repp.tile([P, 512], BF16, tag="rep", name="rep_m")
                    for (i, r) in ((0, rep_s), (1, rep_m)):
                        src = bnc_dr[i, c0:c1]
                        nc.sync.dma_start(
                            out=r[:, 0:w],
                            in_=bass.AP(tensor=src.tensor, offset=src.offset,
                                        ap=[[0, P]] + src.ap))
                    for f in range(8):
                        nc.vector.tensor_mul(xT[:, f, c0:c1], xT[:, f, c0:c1],
                                             rep_s[:, 0:w])
                        nc.vector.tensor_sub(xT[:, f, c0:c1], xT[:, f, c0:c1],
                                             rep_m[:, 0:w])
                    # q projection for this chunk
                    for m in range(4):
                        qp = pjps.tile([P, 512], F32, tag="pj", name="qp")
                        for f in range(8):
                            nc.tensor.matmul(
                                qp[:, 0:w], wq_sb[:, f, m * P:(m + 1) * P],
                                xT[:, f, c0:c1],
                                start=(f == 0), stop=(f == 7))
                        nc.scalar.copy(qT[:, m, c0:c1], qp[:, 0:w])
                    # k projection (only columns < keysc)
                    k1 = min(c1, keysc)
                    if c0 < keysc:
                        kw = k1 - c0
                        for m in range(4):
                            kp = pjps.tile([P, 512], F32, tag="pj", name="kp")
                            for f in range(8):
                                nc.tensor.matmul(
                                    kp[:, 0:kw],
                                    wk_sb[:, f, m * P:(m + 1) * P],
                                    xT[:, f, c0:k1],
                                    start=(f == 0), stop=(f == 7))
                            nc.scalar.copy(kT[:, m, c0:k1], kp[:, 0:kw])
                        # v projection: 128-row tiles within this chunk
                        for rt in range(c0 // P, k1 // P):
                            vp = pjps.tile([P, 512], F32, tag="pj", name="vp")
                            for f in range(8):
                                nc.tensor.matmul(
                                    vp, xT[:, f, rt * P:(rt + 1) * P],
                                    wv_sb[:, f, :],
                                    start=(f == 0), stop=(f == 7))
                            nc.scalar.copy(
                                v_sb[:, rt, :].rearrange(
                                    "p (h c) -> p h c", c=65)[:, :, 0:64],
                                vp.rearrange("p (h c) -> p h c", c=64))
                # null k/v overwrite (slot keysc-1)
                nc.sync.dma_start(
                    out=kT[:, :, keysc - 1:keysc],
                    in_=nullk_sb.rearrange("p (m o) -> p m o", o=1))
                nc.sync.dma_start(out=v_sb[127:128, kcn - 1, :],
                                  in_=nullv_d[:, :])

            # ---------- phase B: l2 norms folded into qT / kT ----------
            with (
                tc.tile_pool(name="bsml", bufs=2) as sml,
                tc.tile_pool(name="bscr", bufs=2) as scr,
                tc.tile_pool(name="l2ps", bufs=4, space="PSUM") as l2ps,
            ):
                for m in range(4):
                    # q side
                    sqq = scr.tile([P, N], BF16, tag="sqq", name="sqq")
                    nc.vector.tensor_mul(sqq, qT[:, m, :], qT[:, m, :])
                    nst = sml.tile([2, N], F32, tag="l2a", name="nstq")
                    for (c0, c1) in _chunks(N):
                        t2 = l2ps.tile([2, 512], F32, tag="l2", name="t2")
                        nc.tensor.matmul(t2[:, 0:c1 - c0], blkdiag,
                                         sqq[:, c0:c1], start=True, stop=True)
                        nc.vector.tensor_copy(nst[:, c0:c1], t2[:, 0:c1 - c0])
                    sd2 = sml.tile([2, N], F32, tag="l2a", name="sd2")
                    nc.scalar.activation(sd2, nst, AF.Sqrt,
                                         bias=eps2_col[0:2, :])
                    nc.vector.reciprocal_approx_fast(nst, sd2)
                    nbf = sml.tile([2, N], BF16, tag="l2b", name="nbfq")
                    nc.vector.tensor_copy(nbf, nst)
                    bq = drp.tile([2, N], BF16, tag="bq", bufs=4,
                                  name=f"bq{m}")
                    nc.sync.dma_start(out=bq, in_=nbf)
                    repq = repp.tile([P, N], BF16, tag="rpq", bufs=2,
                                     name="repq")
                    for h2 in range(2):
                        src = bq[h2, :]
                        nc.sync.dma_start(
                            out=repq[64 * h2:64 * (h2 + 1), :],
                            in_=bass.AP(tensor=src.tensor, offset=src.offset,
                                        ap=[[0, 64]] + src.ap))
                    nc.vector.tensor_scalar(repq, repq, qks_sb[:, 0:1],
                                            None, MUL)
                    nc.vector.tensor_mul(qT[:, m, :], qT[:, m, :], repq)
                    # k side (fold SCALE=8 too)
                    sqk = scr.tile([P, keysc], BF16, tag="sqq", name="sqk")
                    nc.vector.tensor_mul(sqk, kT[:, m, :], kT[:, m, :])
                    nstk = sml.tile([2, keysc], F32, tag="l2a", name="nstk")
                    for (c0, c1) in _chunks(keysc):
                        t2 = l2ps.tile([2, 512], F32, tag="l2", name="t2k")
                        nc.tensor.matmul(t2[:, 0:c1 - c0], blkdiag,
                                         sqk[:, c0:c1], start=True, stop=True)
                        nc.vector.tensor_copy(nstk[:, c0:c1],
                                              t2[:, 0:c1 - c0])
                    sdk = sml.tile([2, keysc], F32, tag="l2a", name="sdk")
                    nc.scalar.activation(sdk, nstk, AF.Sqrt,
                                         bias=eps2_col[0:2, :])
                    nc.vector.reciprocal_approx_fast(nstk, sdk)
                    nbk = sml.tile([2, keysc], BF16, tag="l2b", name="nbk")
                    nc.vector.tensor_scalar(nbk, nstk, SCALE, None, MUL)
                    bk = drp.tile([2, keysc], BF16, tag="bk", bufs=4,
                                  name=f"bk{m}")
                    nc.sync.dma_start(out=bk, in_=nbk)
                    repk = repp.tile([P, keysc], BF16, tag="rpq", bufs=2,
                                     name="repk")
                    for h2 in range(2):
                        src = bk[h2, :]
                        nc.sync.dma_start(
                            out=repk[64 * h2:64 * (h2 + 1), :],
                            in_=bass.AP(tensor=src.tensor, offset=src.offset,
                                        ap=[[0, 64]] + src.ap))
                    nc.vector.tensor_mul(kT[:, m, :], kT[:, m, :], repk)

            # ---------- phase C: attention ----------
            with (
                tc.tile_pool(name="accp", bufs=2, space="PSUM") as accp,
                tc.tile_pool(name="simp", bufs=2, space="PSUM") as simp,
                tc.tile_pool(name="expp", bufs=3) as expp,
                tc.tile_pool(name="omp", bufs=3) as omp,
            ):
                for m in range(4):
                    for rc in range(2):
                        ops = [accp.tile([65, 1024], F32, tag="acc",
                                         name=f"av{i}") for i in range(2)]
                        for kc in range(kcn):
                            for h2 in range(2):
                                sim = simp.tile([P, 1024], F32, tag="sim",
                                                name="sim")
                                for nh in range(2):
                                    r0 = rc * 1024 + nh * 512
                                    nc.tensor.matmul(
                                        sim[:, nh * 512:(nh + 1) * 512],
                                        kT[64 * h2:64 * (h2 + 1),
                                           m, kc * P:(kc + 1) * P],
                                        qT[64 * h2:64 * (h2 + 1),
                                           m, r0:r0 + 512],
                                        start=True, stop=True)
                                e = expp.tile([P, 1024], BF16, tag="e",
                                              name="e")
                                nc.scalar.activation(
                                    e, sim, AF.Exp,
                                    bias=maskc[:, kc:kc + 1])
                                for nh in range(2):
                                    nc.tensor.matmul(
                                        ops[h2][:, nh * 512:(nh + 1) * 512],
                                        v_sb[:, kc,
                                             (2 * m + h2) * 65:
                                             (2 * m + h2 + 1) * 65],
                                        e[:, nh * 512:(nh + 1) * 512],
                                        start=(kc == 0), stop=(kc == kcn - 1))
                        for h2 in range(2):
                            om = omp.tile([65, 1024], BF16, tag="om",
                                          name="om")
                            with nc.allow_low_precision("bf16 numer/denom"):
                                nc.vector.tensor_copy(om, ops[h2])
                            nc.sync.dma_start(
                                out=oT[64 * h2:64 * (h2 + 1), m,
                                       rc * 1024:(rc + 1) * 1024],
                                in_=om[0:64, :])
                            nc.sync.dma_start(
                                out=den_dr[2 * m + h2:2 * m + h2 + 1,
                                           rc * 1024:(rc + 1) * 1024],
                                in_=om[64:65, :])

            # ---------- phase D: divide + output projection ----------
            with (
                tc.tile_pool(name="dps", bufs=4, space="PSUM") as dps,
                tc.tile_pool(name="dnp", bufs=1) as dnp,
                tc.tile_pool(name="dscr", bufs=3) as scr,
            ):
                den_sb = dnp.tile([8, N], BF16)
                nc.sync.dma_start(out=den_sb, in_=den_dr[:, :])
                denf = dnp.tile([8, N], F32)
                nc.vector.tensor_copy(denf, den_sb)
                recf = dnp.tile([8, N], F32)
                nc.vector.reciprocal_approx_fast(recf, denf)
                recb = dnp.tile([8, N], BF16)
                with nc.allow_low_precision("bf16 recip"):
                    nc.vector.tensor_copy(recb, recf)
                nc.sync.dma_start(out=rec_dr[:, :], in_=recb)
                for m in range(4):
                    repd = repp.tile([P, N], BF16, tag="rpq", bufs=2,
                                     name="repd")
                    for h2 in range(2):
                        src = rec_dr[2 * m + h2, :]
                        nc.sync.dma_start(
                            out=repd[64 * h2:64 * (h2 + 1), :],
                            in_=bass.AP(tensor=src.tensor, offset=src.offset,
                                        ap=[[0, 64]] + src.ap))
                    nc.vector.tensor_mul(oT[:, m, :], oT[:, m, :], repd)
                for rt in range(16):
                    for n2 in range(2):
                        op = dps.tile([P, 512], F32, tag="op", name="op")
                        for m in range(4):
                            nc.tensor.matmul(
                                op, oT[:, m, rt * P:(rt + 1) * P],
                                wo_sb[:, m, n2 * 512:(n2 + 1) * 512],
                                start=(m == 0), stop=(m == 3))
                        sg = scr.tile([P, 512], F32, tag="sg", name="sg")
                        if n2 == 0:
                            nc.scalar.copy(sg, op)
                        else:
                            nc.vector.tensor_copy(sg, op)
                        nc.sync.dma_start(
                            out=out_d[rt * P:(rt + 1) * P,
                                      n2 * 512:(n2 + 1) * 512],
                            in_=sg)

    nc.finalize()
    return nc


_NC = {}


def _get_nc(keysc=KEYSC):
    if keysc not in _NC:
        _NC[keysc] = build_nc(keysc)
    return _NC[keysc]


def _shards(x, context_mask, gamma, wq, wkv, null_kv, q_scale, k_scale, wo,
            keysc):
    kcn = keysc // P
    x = np.asarray(x, np.float32)
    gamma = np.asarray(gamma, np.float32)
    wq_g = (np.asarray(wq, np.float32) * gamma[:, None]).astype(BF)
    wkv_g = np.asarray(wkv, np.float32) * gamma[:, None]
    wk_g = wkv_g[:, :D].astype(BF)
    wv_g = wkv_g[:, D:].astype(BF)
    wo = np.asarray(wo, np.float32)
    null_kv = np.asarray(null_kv, np.float32)
    cm = np.asarray(context_mask)
    qs = np.asarray(q_scale, np.float32)
    ks = np.asarray(k_scale, np.float32)
    qks = np.tile(qs * ks, 2).astype(np.float32)[:, None]  # [128,1]

    maps, perms = [], []
    for c in range(8):
        b, hh = c // 2, c % 2
        sl = slice(hh * IC, (hh + 1) * IC)
        heads = np.arange(HC) + hh * HC
        nk = null_kv[0][heads, 0, :]
        nv = null_kv[1][heads, 0, :]
        nullk = np.ascontiguousarray(
            nk.reshape(4, 2, 64).transpose(1, 2, 0).reshape(P, 4))
        nullv = np.zeros((1, HC * 65), np.float32)
        for h in range(HC):
            nullv[0, h * 65:h * 65 + 64] = nv[h]
            nullv[0, h * 65 + 64] = 1.0
        vis = np.flatnonzero(cm[b])
        perm = np.concatenate([vis, np.flatnonzero(~cm[b])])
        perms.append(perm)
        nvis = len(vis)
        bias = np.zeros((keysc,), np.float32)
        bias[nvis:] = NEG
        bias[keysc - 1] = 0.0        # null key always visible
        maskcol = np.ascontiguousarray(bias.reshape(kcn, P).T)
        xp = x[b][perm]
        maps.append({
            "xT": np.ascontiguousarray(xp.T).astype(BF),
            "wq": np.ascontiguousarray(wq_g[:, sl]),
            "wk": np.ascontiguousarray(wk_g[:, sl]),
            "wv": np.ascontiguousarray(wv_g[:, sl]),
            "wo": np.ascontiguousarray(wo[sl, :]).astype(BF),
            "nullk": nullk.astype(BF),
            "nullv": nullv.astype(BF),
            "maskcol": maskcol,
            "qks": qks,
        })
    return maps, perms


def kernel(x, context_mask, gamma, wq, wkv, null_kv, q_scale, k_scale, wo,
           _trace=False):
    cm = np.asarray(context_mask)
    max_vis = int(cm.sum(axis=1).max())
    # need max_vis visible slots plus the null key at slot keysc-1
    keysc = max(KEYSC, ((max_vis + 1 + P - 1) // P) * P)
    nc = _get_nc(keysc)
    maps, perms = _shards(x, context_mask, gamma, wq, wkv, null_kv,
                          q_scale, k_scale, wo, keysc)
    res = run_bass_kernel_spmd(nc, maps, core_ids=list(range(8)),
                               trace=_trace,
                               tmpdir="/tmp/bass_trace" if _trace else None)
    outs = [np.asarray(res.results[c]["out"], np.float32) for c in range(8)]
    full = np.empty((4, N, D), np.float32)
    for b in range(4):
        full[b, perms[2 * b], :] = outs[2 * b] + outs[2 * b + 1]
    if _trace:
        kernel.last_exec_time_ns = res.exec_time_ns
    return full
